# revision 38
# baseline (speedup 1.0000x reference)
"""Trainium2 Bass kernel for nn_CLSAEncoder (2-layer row-shared ConvLSTM +
incremental self-attention refinement), data-parallel over batch on 8 cores.

Restructuring vs the reference scan:
  - attention weights depend only on the *raw* history V, which depends only
    on the ConvLSTM chain -> each layer = (sequential ConvLSTM scan -> V),
    then dense causal S = V V^T, softmax -> W (== attn output),
    then the refinement recurrence R = V + W R solved densely via
    (I-W)^{-1} = prod_j (I + W^(2^j))  (W strictly lower triangular, nilpotent),
    applied as one dense matmul R = M_inv V.
  - layer 1 consumes R0 frames (via a DRAM round-trip for relayout).

Layouts (per core, 1 batch element):
  - ConvLSTM state h/c: partitions x = ch*4 + rgrp (32), free (r8, col);
    h is column-padded (34 wide) so the K=3 conv taps are free-dim slices.
  - gates PSUM: partitions m = gate*32 + x (gate order i,f,o,g), free (r8,col);
    conv = 4..6 accumulating matmuls with host-precomputed block-diagonal
    lhsT weights; conv bias folded into the activation bias (per-partition).
  - d'-order of flattened state vectors: (ch, rgrp, r8, col); consistent
    everywhere on-chip; permuted to the reference order (r, ch, col) only for
    the final encoder output.
"""

import math
import numpy as np
from contextlib import ExitStack

import concourse.bacc as bacc
import concourse.tile as tile
from concourse import mybir
from concourse.bass_utils import run_bass_kernel_spmd

FP = mybir.dt.float32
BF = mybir.dt.bfloat16
NPBF = mybir.dt.np(mybir.dt.bfloat16)
AF = mybir.ActivationFunctionType
AX = mybir.AxisListType

ROWS, COLS, CH, B, KW = 32, 32, 8, 8, 3
D = ROWS * COLS * CH  # 8192
NEG = -1e30

# gate order in the M layout: f, i, o, g ; reference co blocks: i,f,gg,o.
# sigma(f,i,o) = one aligned op on [0:96) written to SBUF (so the c-multiply
# escapes the gates-bank total order); tanh(g) stays in PSUM on [96:128).
_CO_OF_GATE = (8, 0, 24, 16)


def _m_maps():
    co = np.zeros(128, np.int64)
    rg = np.zeros(128, np.int64)
    for gate in range(4):
        for ch in range(CH):
            for r in range(4):
                m = gate * 32 + ch * 4 + r
                co[m] = _CO_OF_GATE[gate] + ch
                rg[m] = r
    return co, rg


def _build_weight_consts(W0, b0, W1, b1):
    co, rg = _m_maps()
    blk = (rg[None, :] == np.arange(4)[:, None])  # [rgrp, m]

    # wx0[q=(rgrp,k), m] = W0[co(m), 0, k] * delta(rgrp)
    wx0 = np.zeros((12, 128), np.float32)
    for rq in range(4):
        for k in range(KW):
            wx0[rq * 3 + k] = W0[co, 0, k] * blk[rq]

    def h_weights(W, ci0):
        # wh[k][x'=(ch',rgrp), m] = W[co(m), ci0+ch', k] * delta(rgrp)
        wh = np.zeros((KW, 32, 128), np.float32)
        for k in range(KW):
            for chp in range(CH):
                for rq in range(4):
                    wh[k, chp * 4 + rq] = W[co, ci0 + chp, k] * blk[rq]
        return wh

    wh0 = h_weights(W0, 1)
    wr1 = h_weights(W1, 0)
    wh1 = h_weights(W1, 8)
    b0t = b0[co].astype(np.float32).reshape(128, 1)
    b1t = b1[co].astype(np.float32).reshape(128, 1)
    return wx0, wh0, wr1, wh1, b0t, b1t


def _x_patches(xb, T):
    # xb: (T, 1024) -> xp[q=(rgrp,k), t, r8, col] = x[t, 8*rgrp+r8, col+k-1]
    xf = xb.reshape(T, ROWS, COLS)
    xp = np.zeros((12, T, 8, COLS), np.float32)
    for rq in range(4):
        for k in range(KW):
            lo, hi = max(0, 1 - k), min(COLS, COLS + 1 - k)
            xp[rq * 3 + k, :, :, lo:hi] = np.transpose(
                xf[:, rq * 8:(rq + 1) * 8, lo + k - 1:hi + k - 1], (0, 1, 2)
            )
    return xp


def _build_program(T):
    nfact = max(1, math.ceil(math.log2(T)))
    nch = D // 128          # 64 d-chunks for transposes / S contraction
    nsl = D // 512          # 16 slices for the R-apply
    CHUNK = 8               # scan streaming chunk (steps)
    SCALE = 1.0 / math.sqrt(D)

    nc = bacc.Bacc()

    xp_d = nc.declare_dram_parameter("xp", [12, T, 8, COLS], BF, isOutput=False)
    wx0_d = nc.declare_dram_parameter("wx0", [12, 128], BF, isOutput=False)
    wh0_d = nc.declare_dram_parameter("wh0", [KW, 32, 128], BF, isOutput=False)
    wr1_d = nc.declare_dram_parameter("wr1", [KW, 32, 128], BF, isOutput=False)
    wh1_d = nc.declare_dram_parameter("wh1", [KW, 32, 128], BF, isOutput=False)
    b0t_d = nc.declare_dram_parameter("b0t", [128, 1], FP, isOutput=False)
    b1t_d = nc.declare_dram_parameter("b1t", [128, 1], FP, isOutput=False)
    id_d = nc.declare_dram_parameter("ident", [128, 128], FP, isOutput=False)
    mask_d = nc.declare_dram_parameter("mask", [T, T], FP, isOutput=False)

    enc_d = nc.declare_dram_parameter("enc", [T, D], FP, isOutput=True)
    attn_d = [
        nc.declare_dram_parameter("attn0", [T, T], FP, isOutput=True),
        nc.declare_dram_parameter("attn1", [T, T], FP, isOutput=True),
    ]
    hf_d = [
        nc.declare_dram_parameter("h0f", [32, 8, COLS], FP, isOutput=True),
        nc.declare_dram_parameter("h1f", [32, 8, COLS], FP, isOutput=True),
    ]
    cf_d = [
        nc.declare_dram_parameter("c0f", [32, 8, COLS], FP, isOutput=True),
        nc.declare_dram_parameter("c1f", [32, 8, COLS], FP, isOutput=True),
    ]

    with tile.TileContext(nc) as tc, ExitStack() as ctx:
        consts = ctx.enter_context(tc.tile_pool(name="consts", bufs=1))
        state = ctx.enter_context(tc.tile_pool(name="state", bufs=1))
        big = ctx.enter_context(tc.tile_pool(name="big", bufs=1))
        stream = ctx.enter_context(tc.tile_pool(name="stream", bufs=2))
        work = ctx.enter_context(tc.tile_pool(name="work", bufs=3))
        small = ctx.enter_context(tc.tile_pool(name="small", bufs=2))
        ps_g = ctx.enter_context(tc.tile_pool(name="ps_g", bufs=2, space="PSUM"))
        ps_t = ctx.enter_context(tc.tile_pool(name="ps_t", bufs=1, space="PSUM"))
        ps_s = ctx.enter_context(tc.tile_pool(name="ps_s", bufs=1, space="PSUM"))
        ps_v = ctx.enter_context(tc.tile_pool(name="ps_v", bufs=1, space="PSUM"))
        ps_r = ctx.enter_context(tc.tile_pool(name="ps_r", bufs=1, space="PSUM"))
        ps_c = ctx.enter_context(tc.tile_pool(name="ps_c", bufs=2, space="PSUM"))
        dram = ctx.enter_context(tc.tile_pool(name="dram", bufs=1, space="DRAM"))

        # ---- load constants ----
        wx0_sb = consts.tile([12, 128], BF, name="wx0c")
        nc.sync.dma_start(out=wx0_sb, in_=wx0_d[:])
        wh_sb = {}
        for nm, dd in (("wh0", wh0_d), ("wr1", wr1_d), ("wh1", wh1_d)):
            t_ = consts.tile([32, KW, 128], BF, name=nm + "c")
            nc.sync.dma_start(out=t_, in_=dd[:].rearrange("k x m -> x k m"))
            wh_sb[nm] = t_
        b_sb = []
        for nm, dd in (("b0c", b0t_d), ("b1c", b1t_d)):
            t_ = consts.tile([128, 1], FP, name=nm)
            nc.sync.dma_start(out=t_, in_=dd[:])
            b_sb.append(t_)
        id_sb = consts.tile([128, 128], FP, name="idc")
        nc.sync.dma_start(out=id_sb, in_=id_d[:])
        mask_sb = consts.tile([T, T], FP, name="maskc")
        nc.sync.dma_start(out=mask_sb, in_=mask_d[:])

        v_dram = [dram.tile([T, D], FP, name="v0d"), dram.tile([T, D], FP, name="v1d")]
        r0_dram = dram.tile([T, D], BF, name="r0d")

        # ---- sequential ConvLSTM scan for one layer ----
        def scan_layer(layer, vt):
            # fp32 h state is double-buffered so the V-append DMA read never
            # WAR-stalls the next step's h write; the conv consumes a bf16
            # padded copy (hb, also ping-ponged) so the gate matmuls stream
            # at full bf16 rate.
            hs = [state.tile([32, 8, COLS], FP, name=f"h{layer}{i}") for i in range(2)]
            hbs = [state.tile([32, 8, COLS + 2], BF, name=f"hb{layer}{i}")
                   for i in range(2)]
            c = state.tile([32, 8, COLS], FP, name=f"c{layer}")
            for i in range(2):
                nc.vector.memset(hbs[i][:], 0.0)
            nc.vector.memset(c[:], 0.0)
            bt = b_sb[layer]
            xsb = None
            for t in range(T):
                h_new = hs[t % 2]
                hb_prev = hbs[(t + 1) % 2]   # holds h(t-1), zero at t=0
                hb_new = hbs[t % 2]
                s = t % CHUNK
                if s == 0:
                    if layer == 0:
                        xsb = stream.tile([12, CHUNK, 8, COLS], BF, tag="stream")
                        nc.sync.dma_start(out=xsb, in_=xp_d[:, t:t + CHUNK])
                    else:
                        xsb = stream.tile([32, CHUNK, 8, COLS + 2], BF, tag="stream")
                        nc.vector.memset(xsb[:, :, :, 0:1], 0.0)
                        nc.vector.memset(xsb[:, :, :, 33:34], 0.0)
                        for s2 in range(CHUNK):
                            nc.sync.dma_start(
                                out=xsb[:, s2, :, 1:33],
                                in_=r0_dram[t + s2, :].rearrange(
                                    "(x a b) -> x a b", x=32, a=8
                                ),
                            )
                g = ps_g.tile([128, 8, COLS], FP, tag="gates")
                if layer == 0:
                    mms = [(wx0_sb, xsb[:, s])]
                    mms += [(wh_sb["wh0"][:, k], hb_prev[:, :, k:k + 32])
                            for k in range(KW)]
                else:
                    mms = [(wh_sb["wr1"][:, k], xsb[:, s, :, k:k + 32])
                           for k in range(KW)]
                    mms += [(wh_sb["wh1"][:, k], hb_prev[:, :, k:k + 32])
                            for k in range(KW)]
                for i, (lhsT, rhs) in enumerate(mms):
                    nc.tensor.matmul(g, lhsT, rhs, start=(i == 0), stop=(i == len(mms) - 1))

                # sigma(i,f,o) in place in PSUM (one aligned 96-partition op);
                # tanh(g) lands in SBUF at base 96 so later DVE ops always mix
                # one PSUM operand with one SBUF operand (differing bases OK).
                sg = work.tile([128, 8, COLS], FP, tag="sg")
                nc.scalar.activation(sg[0:96], g[0:96], AF.Sigmoid, bias=bt[0:96])
                nc.scalar.activation(g[96:128], g[96:128], AF.Tanh, bias=bt[96:128])
                t1 = work.tile([32, 8, COLS], FP, tag="t1")
                nc.vector.tensor_mul(c[:], sg[0:32], c[:])
                nc.vector.tensor_mul(t1, sg[32:64], g[96:128])
                nc.vector.tensor_add(c[:], c[:], t1)
                tch = ps_c.tile([32, 8, COLS], FP, tag="tc")
                nc.scalar.activation(tch, c[:], AF.Tanh)
                # bf16 state for the next conv comes straight off the gate
                # multiply (chain); the fp32 copy for the V history is a
                # second multiply consumed only by the off-chain DMA.
                nc.vector.tensor_mul(hb_new[:, :, 1:33], sg[64:96], tch)
                nc.vector.tensor_mul(h_new[:], sg[64:96], tch)
                nc.gpsimd.dma_start(
                    out=v_dram[layer][t, :].rearrange("(x a b) -> x a b", x=32, a=8),
                    in_=h_new[:],
                )
                # incremental VT build: transpose this step's h into the
                # (d-on-partitions) history right away, using scan-idle PE/DVE
                # slack instead of a serial stage in the attention phase.
                # d' = x*256 + f, f = r8*32+col; half hi covers f in
                # [128*hi, 128*hi+128) -> chunk c = 2*x + hi, p = f - 128*hi.
                vt4 = vt.rearrange("p (q two) t -> p q two t", two=2)
                for hi in range(2):
                    tpp = ps_t.tile([128, 32], FP, tag="tp")
                    nc.tensor.transpose(tpp, h_new[:, 4 * hi:4 * hi + 4, :],
                                        id_sb[0:32, 0:32])
                    nc.vector.tensor_copy(vt4[:, :, hi, t], tpp)
            return hs[(T - 1) % 2], c

        # ---- dense attention + refinement solve for one layer ----
        def attention(layer, vt):
            vn = big.tile([T, D], FP, tag="vn")
            nc.sync.dma_start(out=vn, in_=v_dram[layer][:])
            S = ps_s.tile([T, T], FP, tag="S")
            for cc in range(nch):
                nc.tensor.matmul(S, vt[:, cc, :], vt[:, cc, :],
                                 start=(cc == 0), stop=(cc == nch - 1))
            wm = state.tile([T, T], FP, name="wm")
            nc.vector.tensor_scalar_mul(wm, S, float(SCALE))
            nc.vector.tensor_add(wm, wm, mask_sb)
            mx = small.tile([T, 1], FP, tag="mx")
            nc.vector.reduce_max(mx, wm, axis=AX.X)
            mxn = small.tile([T, 1], FP, tag="mxn")
            nc.vector.tensor_scalar_mul(mxn, mx, -1.0)
            nc.scalar.activation(wm, wm, AF.Exp, bias=mxn)
            sm = small.tile([T, 1], FP, tag="sm")
            nc.vector.reduce_sum(sm, wm, axis=AX.X)
            rs = small.tile([T, 1], FP, tag="rs")
            nc.vector.reciprocal(rs, sm)
            nc.vector.tensor_scalar_mul(wm, wm, rs)
            nc.vector.memset(wm[0:1, :], 0.0)
            nc.sync.dma_start(out=attn_d[layer][:], in_=wm)

            # (I-W)^-1 via nilpotent product; track Q,Q^T,P,P^T so every
            # matmul has its lhsT available pre-transposed.
            qtp = ps_t.tile([T, T], FP, tag="tp")
            nc.tensor.transpose(qtp, wm, id_sb[0:T, 0:T])
            Q = state.tile([T, T], FP, name="Q")
            nc.vector.tensor_copy(Q, wm)
            QT = state.tile([T, T], FP, name="QT")
            nc.vector.tensor_copy(QT, qtp)
            P = state.tile([T, T], FP, name="P")
            nc.vector.tensor_add(P, id_sb[0:T, 0:T], wm)
            PT = state.tile([T, T], FP, name="PT")
            nc.vector.tensor_add(PT, id_sb[0:T, 0:T], qtp)
            for j in range(1, nfact):
                sv = ps_v.tile([T, 4, T], FP, tag="sv")
                last = j == nfact - 1
                # square first: Q <- N^(2^j)
                nc.tensor.matmul(sv[:, 0], QT, Q, start=True, stop=True)
                nc.tensor.matmul(sv[:, 1], Q, QT, start=True, stop=True)
                nc.vector.tensor_copy(Q, sv[:, 0])
                nc.vector.tensor_copy(QT, sv[:, 1])
                # then apply the factor (I + N^(2^j)) to P / PT
                nc.tensor.matmul(sv[:, 3], P, QT, start=True, stop=True)
                if not last:
                    nc.tensor.matmul(sv[:, 2], QT, P, start=True, stop=True)
                    nc.vector.tensor_add(P, P, sv[:, 2])
                nc.vector.tensor_add(PT, PT, sv[:, 3])
            # R = M_inv @ V  (lhsT = M_inv^T = PT). Layer 0's R only feeds the
            # layer-1 conv input, so it is downcast to bf16 on the PSUM copy.
            rdt = BF if layer == 0 else FP
            rn = big.tile([T, D], rdt, tag=f"rn{layer}")
            for n in range(nsl):
                rp = ps_r.tile([T, 512], FP, tag="rp")
                nc.tensor.matmul(rp, PT, vn[:, n * 512:(n + 1) * 512],
                                 start=True, stop=True)
                nc.vector.tensor_copy(rn[:, n * 512:(n + 1) * 512], rp)
            return rn

        vt0 = big.tile([128, nch, T], FP, tag="vt")
        h0, c0 = scan_layer(0, vt0)
        r0 = attention(0, vt0)
        nc.sync.dma_start(out=r0_dram[:], in_=r0)
        vt1 = big.tile([128, nch, T], FP, tag="vt")
        h1, c1 = scan_layer(1, vt1)
        r1 = attention(1, vt1)

        # encoder output: permute d' = (ch, rgrp, r8, col) -> (rgrp, r8, ch, col)
        rp_out = big.tile([T, 4, 8, CH, COLS], FP, tag="vt")
        nc.vector.tensor_copy(
            rp_out, r1[:].rearrange("t (ch rg a b) -> t rg a ch b", ch=CH, rg=4, a=8)
        )
        nc.sync.dma_start(out=enc_d[:].rearrange("t (rg a ch b) -> t rg a ch b",
                                                 rg=4, a=8, ch=CH), in_=rp_out)

        for layer, (h, c) in enumerate(((h0, c0), (h1, c1))):
            nc.sync.dma_start(out=hf_d[layer][:], in_=h[:])
            nc.sync.dma_start(out=cf_d[layer][:], in_=c[:])

    nc.finalize()
    return nc


_PROGRAM_CACHE = {}


def _get_program(T):
    if T not in _PROGRAM_CACHE:
        _PROGRAM_CACHE[T] = _build_program(T)
    return _PROGRAM_CACHE[T]


def _unpack_state(a):
    # [32=(ch*4+rgrp), 8, 32] -> (ROWS, CH, COLS)
    return a.reshape(CH, 4, 8, COLS).transpose(1, 2, 0, 3).reshape(ROWS, CH, COLS)


def run(x_flat, W0, b0, W1, b1, trace=False):
    x_flat = np.asarray(x_flat, np.float32)
    W0 = np.asarray(W0, np.float32)
    b0 = np.asarray(b0, np.float32)
    W1 = np.asarray(W1, np.float32)
    b1 = np.asarray(b1, np.float32)
    nb, T = x_flat.shape[0], x_flat.shape[1]
    assert nb == B and x_flat.shape[2] == ROWS * COLS

    wx0, wh0, wr1, wh1, b0t, b1t = _build_weight_consts(W0, b0, W1, b1)
    ident = np.eye(128, dtype=np.float32)
    mask = np.where(np.arange(T)[None, :] < np.arange(T)[:, None], 0.0, NEG).astype(
        np.float32
    )
    base = {"wx0": wx0.astype(NPBF), "wh0": wh0.astype(NPBF),
            "wr1": wr1.astype(NPBF), "wh1": wh1.astype(NPBF),
            "b0t": b0t, "b1t": b1t, "ident": ident, "mask": mask}
    in_maps = [dict(base, xp=_x_patches(x_flat[b], T).astype(NPBF))
               for b in range(B)]

    nc = _get_program(T)
    res = run_bass_kernel_spmd(nc, in_maps, list(range(B)), trace=trace)

    enc = np.stack([res.results[b]["enc"] for b in range(B)])  # (B, T, D)
    attn0 = np.stack([res.results[b]["attn0"] for b in range(B)])
    attn1 = np.stack([res.results[b]["attn1"] for b in range(B)])
    states = []
    for nm in ("h0f", "c0f", "h1f", "c1f"):
        states.append(np.stack([_unpack_state(res.results[b][nm]) for b in range(B)]))
    h0f, c0f, h1f, c1f = states
    return (enc, (h0f, c0f, h1f, c1f), attn0, attn1), res


def kernel(x_flat, W0, b0, W1, b1):
    out, _ = run(x_flat, W0, b0, W1, b1)
    return out
